# revision 1
# baseline (speedup 1.0000x reference)
"""Bloom-style attention block (QKV proj + ALiBi causal attention + dense) on 8
Trainium2 NeuronCores, tensor-parallel over heads (4 heads per core), partial
dense outputs all-reduced on the host.

Layout strategy: the host pre-transposes activations/weights into
contraction-major ("T") layouts so the device needs zero on-chip transposes:
  - xT      [128, 32, 4096]  : x^T tiled   (h = 128*ht + p, t free)
  - wqT/wkT [128, 32, 512]   : W^T tiles   (o = 4 heads x 128 free)
  - scores are computed transposed  sT[k, q]  so softmax sums over the
    partition axis reduce via a ones-vector matmul, p^T feeds attn@v
    directly, and the ALiBi bias+causal mask fold into one precomputed
    multiplicative tile  B = exp(slope*(j-i)) * (j<=i).
Matmul dtypes: fp32r (full PE rate at N>=512, ~tf32 accuracy) for the QKV
projection and scores; bf16 for p/v/dense.
"""

import sys

sys.path.insert(0, "/opt/trn_rl_repo")

import math

import ml_dtypes
import numpy as np

B, S, H, NH = 2, 2048, 4096, 32
HD = H // NH          # 128
N_CORES = 8
HPC = NH // N_CORES   # 4 heads per core
T = B * S             # 4096 tokens
SCALE = HD ** -0.5

F32 = np.float32
BF16 = ml_dtypes.bfloat16


def _alibi_slopes(n: int) -> np.ndarray:
    cp2 = 2 ** math.floor(math.log2(n))
    base = 2.0 ** (-(2.0 ** (-(math.log2(cp2) - 3))))
    slopes = base ** np.arange(1, cp2 + 1, dtype=np.float64)
    if cp2 != n:
        extra_base = 2.0 ** (-(2.0 ** (-(math.log2(2 * cp2) - 3))))
        rem = min(cp2, n - cp2)
        extra = extra_base ** np.arange(1, 1 + 2 * rem, 2, dtype=np.float64)
        slopes = np.concatenate([slopes, extra])
    return slopes.astype(np.float64)


def heads_for_core(c: int) -> list[int]:
    # Interleaved so each head-slot j holds heads {8j..8j+7} across cores:
    # keeps the SPMD program uniform if per-slot ALiBi tile skipping is used.
    return [8 * j + c for j in range(HPC)]


# Per head-slot j, per q-block q0 (512 queries), the k-tiles (128 keys) kept.
# B-tile entries decay as exp(slope*rel); a tile whose largest rel is below
# ~-75/slope contributes < e^-60 relative weight and its B tile is exactly 0
# in bf16, so skipping it changes nothing.  Slot j's smallest slope is
# slope(8j+7); cores share one program so the slot keep-list uses that bound.
_SLOPES = _alibi_slopes(NH)


def _keep_k_tiles(j: int, q0: int) -> list[int]:
    min_slope = _SLOPES[8 * j + 7]
    keep = []
    for kt in range(4 * q0 + 4):
        max_rel = 128 * kt + 127 - 512 * q0  # max over tile of (k - q)
        if max_rel >= 0 or min_slope * max_rel > -75.0:
            keep.append(kt)
    return keep


def _build_program(trips: int = 1, stages: str = "123"):
    from concourse import bacc
    import concourse.tile as tile
    import concourse.mybir as mybir

    f32 = mybir.dt.float32
    f32r = mybir.dt.float32r
    bf16 = mybir.dt.bfloat16
    AF = mybir.ActivationFunctionType
    MULT = mybir.AluOpType.mult

    nc = bacc.Bacc("TRN2", target_bir_lowering=False, debug=False)

    xT = nc.dram_tensor("xT", [128, 32, T], f32r, kind="ExternalInput")
    wqkT = nc.dram_tensor("wqkT", [128, 32, 1024], f32r, kind="ExternalInput")
    wvT = nc.dram_tensor("wvT", [128, 32, 512], f32r, kind="ExternalInput")
    bqk = nc.dram_tensor("bqk", [128, 8], f32, kind="ExternalInput")
    bv = nc.dram_tensor("bv", [1, 512], f32r, kind="ExternalInput")
    btil = nc.dram_tensor("btil", [HPC, 128, 16, 512], bf16, kind="ExternalInput")
    ones_r = nc.dram_tensor("ones_r", [1, 128], f32r, kind="ExternalInput")
    ones_c = nc.dram_tensor("ones_c", [128, 1], bf16, kind="ExternalInput")
    wdT = nc.dram_tensor("wdT", [128, HPC, H], bf16, kind="ExternalInput")
    out = nc.dram_tensor("out", [T, H], f32, kind="ExternalOutput")

    qkT_sp = nc.dram_tensor("qkT_sp", [128, 8, T], f32r)   # scratch: q,k feature-major
    v_sp = nc.dram_tensor("v_sp", [128, 32, 512], bf16)    # scratch: v token-major

    TBLK = 512  # stage-1 token-block (fp32r needs moving free dim >= 256)
    NBLK = T // TBLK

    with tile.TileContext(nc) as tc:
        with tc.tile_pool(name="const", bufs=1) as pconst:
            ones_col = pconst.tile([128, 1], bf16)
            nc.sync.dma_start(out=ones_col[:, :], in_=ones_c[:, :])
            ones_row = pconst.tile([1, 128], f32r)
            nc.sync.dma_start(out=ones_row[:, :], in_=ones_r[:, :])
            bqk_sb = pconst.tile([128, 8], f32)
            nc.sync.dma_start(out=bqk_sb[:, :], in_=bqk[:, :])
            bv_sb = pconst.tile([1, 512], f32r)
            nc.sync.dma_start(out=bv_sb[:, :], in_=bv[:, :])

            def _stages():
              # ---------------- stage 1: QKV projections ----------------
              if "1" in stages:
               # pass A: q and k fused (one xT stream, W_qk resident)
               with tc.tile_pool(name="w1a", bufs=1) as pw, \
                    tc.tile_pool(name="x1a", bufs=2) as px, \
                    tc.tile_pool(name="ev1a", bufs=4) as pev, \
                    tc.tile_pool(name="ps1a", bufs=4, space="PSUM") as pps:
                   w_sb = pw.tile([128, 32, 1024], f32r)
                   nc.sync.dma_start(out=w_sb[:, :, :], in_=wqkT[:, :, :])
                   TA = 256
                   for tb in range(T // TA):
                       x_sb = px.tile([128, 32, TA], f32r, tag="x_a")
                       nc.sync.dma_start(
                           out=x_sb[:, :, :], in_=xT[:, :, TA * tb:TA * (tb + 1)]
                       )
                       for ji in range(8):  # 4 q head-slots then 4 k head-slots
                           ps = pps.tile([128, TA], f32, tag="ps1")
                           for h in range(32):
                               nc.tensor.matmul(
                                   ps[:, :],
                                   w_sb[:, h, 128 * ji:128 * (ji + 1)],
                                   x_sb[:, h, :],
                                   start=(h == 0), stop=(h == 31),
                               )
                           ev = pev.tile([128, TA], f32r, tag="ev_qk")
                           nc.scalar.activation(
                               ev[:, :], ps[:, :], AF.Identity,
                               bias=bqk_sb[:, ji:ji + 1],
                           )
                           nc.sync.dma_start(
                               out=qkT_sp[:, ji, TA * tb:TA * (tb + 1)],
                               in_=ev[:, :],
                           )
               # pass B: v, token-major
               with tc.tile_pool(name="w1b", bufs=1) as pw, \
                    tc.tile_pool(name="x1b", bufs=2) as px, \
                    tc.tile_pool(name="ev1b", bufs=4) as pev, \
                    tc.tile_pool(name="ps1b", bufs=4, space="PSUM") as pps:
                   w_sb = pw.tile([128, 32, 512], f32r)
                   nc.sync.dma_start(out=w_sb[:, :, :], in_=wvT[:, :, :])
                   for tb in range(T // 512):
                       x_sb = px.tile([128, 32, 512], f32r, tag="x_b")
                       nc.sync.dma_start(
                           out=x_sb[:, :, :], in_=xT[:, :, 512 * tb:512 * (tb + 1)]
                       )
                       for tt in range(4):
                           ps = pps.tile([128, 512], f32, tag="ps1v")
                           nc.tensor.matmul(  # bias row outer product
                               ps[:, :], ones_row[:, :], bv_sb[:, :],
                               start=True, stop=False,
                           )
                           for h in range(32):
                               nc.tensor.matmul(
                                   ps[:, :],
                                   x_sb[:, h, 128 * tt:128 * (tt + 1)],
                                   w_sb[:, h, :],
                                   start=False, stop=(h == 31),
                               )
                           ev = pev.tile([128, 512], bf16, tag="ev_v")
                           nc.scalar.copy(ev[:, :], ps[:, :])
                           nc.sync.dma_start(
                               out=v_sp[:, 4 * tb + tt, :], in_=ev[:, :]
                           )

              # ---------------- stage 2: attention ----------------
              from contextlib import ExitStack
              _stk = ExitStack()
              ppersist = _stk.enter_context(tc.tile_pool(name="persist", bufs=1))
              attnoutT = ppersist.tile([128, HPC, T], bf16)  # lives st2->st3
              if "2" not in stages and "3" in stages:
                  # stage-3-only build: fill attnoutT from btil so it is written
                  nc.sync.dma_start(out=attnoutT[:, 0:2, :], in_=btil[0, :, :, :].rearrange("p a b -> p (a b)").rearrange("p (c t) -> p c t", c=2))
                  nc.sync.dma_start(out=attnoutT[:, 2:4, :], in_=btil[1, :, :, :].rearrange("p a b -> p (a b)").rearrange("p (c t) -> p c t", c=2))
              if "2" in stages:
               with tc.tile_pool(name="wd3", bufs=1) as pwd, \
                    tc.tile_pool(name="o3", bufs=4) as po, \
                    tc.tile_pool(name="ps3", bufs=2, space="PSUM") as pp3, \
                    tc.tile_pool(name="qk2", bufs=1) as pqk, \
                   tc.tile_pool(name="v2", bufs=1) as pv, \
                   tc.tile_pool(name="bt2", bufs=2) as pbt, \
                   tc.tile_pool(name="p2", bufs=8) as pp, \
                   tc.tile_pool(name="bc2", bufs=1) as pbc, \
                   tc.tile_pool(name="rr2", bufs=1) as prr, \
                   tc.tile_pool(name="pso", bufs=2, space="PSUM") as ppo, \
                   tc.tile_pool(name="psr", bufs=1, space="PSUM") as ppr, \
                   tc.tile_pool(name="pss", bufs=3, space="PSUM") as pp_s, \
                   tc.tile_pool(name="psb", bufs=1, space="PSUM") as ppb:
                  wd_sb = pwd.tile([128, HPC, H], bf16)
                  nc.sync.dma_start(out=wd_sb[:, :, :], in_=wdT[:, :, :])
                  for b in range(B):
                      qk_sb = pqk.tile([128, 8, S], f32r, tag="qk_sb")
                      nc.sync.dma_start(
                          out=qk_sb[:, :, :], in_=qkT_sp[:, :, S * b:S * (b + 1)]
                      )
                      v_sb = pv.tile([128, 16, 512], bf16, tag="v_sb")
                      nc.sync.dma_start(
                          out=v_sb[:, :, :], in_=v_sp[:, 16 * b:16 * (b + 1), :]
                      )
                      for j in range(HPC):
                          bt_sb = pbt.tile([128, 16, 512], bf16, tag="bt_sb")
                          nc.sync.dma_start(out=bt_sb[:, :, :], in_=btil[j, :, :, :])
                          for q0 in range(4):
                              keep = _keep_k_tiles(j, q0)
                              out_ps = ppo.tile([128, 512], f32, tag="out_ps")
                              rs_ps = ppr.tile([1, 512], f32, tag="rs_ps")
                              # software pipeline: run scores+exp+mask LA tiles
                              # ahead so the PE never waits on ACT/DVE
                              LA = 3
                              pending = []

                              def _produce(kt):
                                  s_ps = pp_s.tile([128, 512], f32, tag="s_ps")
                                  nc.tensor.matmul(
                                      s_ps[:, :],
                                      qk_sb[:, 4 + j, 128 * kt:128 * (kt + 1)],
                                      qk_sb[:, j, 512 * q0:512 * (q0 + 1)],
                                      start=True, stop=True,
                                  )
                                  p0 = pp.tile([128, 512], bf16, tag="p0")
                                  nc.scalar.activation(
                                      p0[:, :], s_ps[:, :], AF.Exp, scale=SCALE
                                  )
                                  pT = pp.tile([128, 512], bf16, tag="pT")
                                  nc.vector.tensor_tensor(
                                      pT[:, :], p0[:, :],
                                      bt_sb[:, kt - 4 * q0 + 12, :], op=MULT,
                                  )
                                  return pT

                              def _consume(i, kt, pT):
                                  first, last = (i == 0), (i == len(keep) - 1)
                                  nc.tensor.matmul(
                                      out_ps[:, :],
                                      v_sb[:, kt, 128 * j:128 * (j + 1)],
                                      pT[:, :], start=first, stop=last,
                                  )
                                  nc.tensor.matmul(
                                      rs_ps[:, :], ones_col[:, :], pT[:, :],
                                      start=first, stop=last,
                                  )

                              for i, kt in enumerate(keep):
                                  pending.append((i, kt, _produce(kt)))
                                  if len(pending) > LA:
                                      _consume(*pending.pop(0))
                              for item in pending:
                                  _consume(*item)
                              rr = prr.tile([1, 512], f32r, tag="rr")
                              with nc.allow_low_precision(reason="f32r is f32 bits"):
                                  nc.vector.reciprocal(rr[:, :], rs_ps[:, :])
                              bc_ps = pp_s.tile([128, 512], f32, tag="s_ps")
                              nc.tensor.matmul(
                                  bc_ps[:, :], ones_row[:, :], rr[:, :],
                                  start=True, stop=True,
                              )
                              bc_sb = pbc.tile([128, 512], f32, tag="bc_sb")
                              nc.scalar.copy(bc_sb[:, :], bc_ps[:, :])
                              tpos = S * b + 512 * q0
                              nc.vector.tensor_tensor(
                                  attnoutT[:, j, tpos:tpos + 512],
                                  out_ps[:, :], bc_sb[:, :], op=MULT,
                              )
                      for tt in range(16 * b, 16 * (b + 1)):
                          for ob in range(8):
                              ps3 = pp3.tile([128, 512], f32, tag="ps3")
                              for j in range(HPC):
                                  nc.tensor.matmul(
                                      ps3[:, :],
                                      attnoutT[:, j, 128 * tt:128 * (tt + 1)],
                                      wd_sb[:, j, 512 * ob:512 * (ob + 1)],
                                      start=(j == 0), stop=(j == HPC - 1),
                                  )
                              o_sb = po.tile([128, 512], f32, tag="o_sb")
                              nc.scalar.copy(o_sb[:, :], ps3[:, :])
                              nc.sync.dma_start(
                                  out=out[128 * tt:128 * (tt + 1),
                                          512 * ob:512 * (ob + 1)],
                                  in_=o_sb[:, :],
                              )

              # ---------------- stage 3: dense (partial over this core's heads) --
              # (emitted per batch inside stage 2 so dense MMs can fill PE gaps)

              _stk.close()

            if trips == 1:
                _stages()
            else:
                with tc.For_i(0, trips, 1):
                    _stages()

    nc.compile()
    return nc


_NC_CACHE = None


def _get_program():
    global _NC_CACHE
    if _NC_CACHE is None:
        _NC_CACHE = _build_program()
    return _NC_CACHE


def _feature_major(w_rows: np.ndarray) -> np.ndarray:
    # [512, H] weight rows -> [128, 32, 512] contraction-major tiles
    return np.ascontiguousarray(
        w_rows.T.reshape(32, 128, 512).transpose(1, 0, 2), dtype=F32
    )


def make_core_inputs(hidden_states, w_qkv, b_qkv, w_dense):
    """Shard + relayout full inputs into the 8 per-core input maps."""
    x = np.asarray(hidden_states, dtype=F32).reshape(T, H)
    xT_full = np.ascontiguousarray(
        x.T.reshape(32, 128, T).transpose(1, 0, 2), dtype=F32
    )
    w_qkv = np.asarray(w_qkv, dtype=F32)
    b_qkv = np.asarray(b_qkv, dtype=F32)
    w_dense = np.asarray(w_dense, dtype=F32)

    in_maps = []
    for c in range(N_CORES):
        heads = heads_for_core(c)
        rows = np.concatenate([np.arange(HD * h, HD * (h + 1)) for h in heads])
        wqk = np.ascontiguousarray(
            np.concatenate([w_qkv[rows], w_qkv[H + rows]]).T
            .reshape(32, 128, 1024).transpose(1, 0, 2), dtype=F32)
        wv = _feature_major(w_qkv[2 * H + rows])
        bq = b_qkv[rows].reshape(HPC, 128)
        bk = b_qkv[H + rows].reshape(HPC, 128)
        bqk_c = np.stack([*bq, *bk], axis=1).astype(F32)  # [128, 8]
        bv_c = b_qkv[2 * H + rows].reshape(1, 512).astype(F32)

        btil_c = np.zeros((HPC, 128, 16, 512), dtype=BF16)
        kl = np.arange(128)[:, None]
        ql = np.arange(512)[None, :]
        for j, h in enumerate(heads):
            slope = _SLOPES[h]
            for idx in range(16):
                rel = (128 * (idx - 12) + kl - ql).astype(np.float64)
                tilev = np.where(rel <= 0, np.exp(slope * rel), 0.0)
                btil_c[j, :, idx, :] = tilev.astype(BF16)

        wd_c = np.stack(
            [np.ascontiguousarray(w_dense[:, HD * h:HD * (h + 1)].T) for h in heads],
            axis=1,
        ).astype(BF16)  # [128, HPC, H]

        in_maps.append({
            "ones_r": np.ones((1, 128), dtype=F32),
            "ones_c": np.ones((128, 1), dtype=BF16),
            "xT": xT_full,
            "wqkT": wqk, "wvT": wv,
            "bqk": bqk_c, "bv": bv_c,
            "btil": btil_c, "wdT": wd_c,
        })
    return in_maps


def kernel(hidden_states, w_qkv, b_qkv, w_dense, b_dense):
    from concourse.bass_utils import run_bass_kernel_spmd

    nc = _get_program()
    in_maps = make_core_inputs(hidden_states, w_qkv, b_qkv, w_dense)
    res = run_bass_kernel_spmd(nc, in_maps, core_ids=list(range(N_CORES)))
    acc = np.zeros((T, H), dtype=np.float32)
    for c in range(N_CORES):
        acc += res.results[c]["out"]
    acc += np.asarray(b_dense, dtype=np.float32)[None, :]
    return acc.reshape(B, S, H).astype(np.float32)



# revision 5
# speedup vs baseline: 1.1213x; 1.1213x over previous
"""Bloom-style attention block (QKV proj + ALiBi causal attention + dense) on 8
Trainium2 NeuronCores, tensor-parallel over heads (4 heads per core), partial
dense outputs all-reduced on the host.

v2: all matmuls in bf16 (fp32r's 4-byte self-loading weight path serialized a
~107ns LDWEIGHTS into every projection matmul; bf16 gets FWL loads that hide
under the matmul stream).  Single fused pass over x computes q,k,v; v and the
attention output stay SBUF-resident; q/k round-trip DRAM in bf16.  Scores are
computed transposed sT[k,q] so softmax sums reduce via a ones-vector matmul and
pT feeds attn@v directly; ALiBi bias+causal mask fold into one precomputed
multiplicative tile B = exp(slope*(j-i)) * (j<=i).  The q-side weights/bias are
pre-scaled by 1/sqrt(HD) on the host so the exp needs no extra scale.
"""

import sys

sys.path.insert(0, "/opt/trn_rl_repo")

import math

import ml_dtypes
import numpy as np

B, S, H, NH = 2, 2048, 4096, 32
HD = H // NH          # 128
N_CORES = 8
HPC = NH // N_CORES   # 4 heads per core
T = B * S             # 4096 tokens
SCALE = HD ** -0.5

F32 = np.float32
BF16 = ml_dtypes.bfloat16


def _alibi_slopes(n: int) -> np.ndarray:
    cp2 = 2 ** math.floor(math.log2(n))
    base = 2.0 ** (-(2.0 ** (-(math.log2(cp2) - 3))))
    slopes = base ** np.arange(1, cp2 + 1, dtype=np.float64)
    if cp2 != n:
        extra_base = 2.0 ** (-(2.0 ** (-(math.log2(2 * cp2) - 3))))
        rem = min(cp2, n - cp2)
        extra = extra_base ** np.arange(1, 1 + 2 * rem, 2, dtype=np.float64)
        slopes = np.concatenate([slopes, extra])
    return slopes.astype(np.float64)


def heads_for_core(c: int) -> list[int]:
    # Interleaved so each head-slot j holds heads {8j..8j+7} across cores:
    # keeps the SPMD program uniform while letting the per-slot ALiBi tile
    # skip-list use the slot's smallest slope (head 8j+7).
    return [8 * j + c for j in range(HPC)]


# Per head-slot j, per q-block q0 (512 queries), the k-tiles (128 keys) kept.
# A tile whose best (largest) rel has slope*rel <= -20 contributes < 2e-9
# relative softmax weight per key (< 4e-6 summed) -- far below the 2e-2 gate.
_SLOPES = _alibi_slopes(NH)


def _keep_k_tiles(j: int, q0: int) -> list[int]:
    min_slope = _SLOPES[8 * j + 7]
    keep = []
    for kt in range(4 * q0 + 4):
        max_rel = 128 * kt + 127 - 512 * q0  # max over tile of (k - q)
        if max_rel >= 0 or min_slope * max_rel > -20.0:
            keep.append(kt)
    return keep


def _build_program(trips: int = 1, stages: str = "123"):
    from concourse import bacc
    import concourse.tile as tile
    import concourse.mybir as mybir

    f32 = mybir.dt.float32
    bf16 = mybir.dt.bfloat16
    AF = mybir.ActivationFunctionType
    MULT = mybir.AluOpType.mult

    nc = bacc.Bacc("TRN2", target_bir_lowering=False, debug=False)

    xT = nc.dram_tensor("xT", [128, 32, T], bf16, kind="ExternalInput")
    wqkT = nc.dram_tensor("wqkT", [128, 32, 1024], bf16, kind="ExternalInput")
    wvT = nc.dram_tensor("wvT", [128, 32, 512], bf16, kind="ExternalInput")
    bqk = nc.dram_tensor("bqk", [128, 8], f32, kind="ExternalInput")
    bv = nc.dram_tensor("bv", [1, 512], bf16, kind="ExternalInput")
    btil = nc.dram_tensor("btil", [HPC, 128, 16, 512], bf16, kind="ExternalInput")
    ones_r = nc.dram_tensor("ones_r", [1, 128], bf16, kind="ExternalInput")
    ones_c = nc.dram_tensor("ones_c", [128, 1], bf16, kind="ExternalInput")
    wdT = nc.dram_tensor("wdT", [128, HPC, H], bf16, kind="ExternalInput")
    out = nc.dram_tensor("out", [T, H], bf16, kind="ExternalOutput")

    qkT_sp = nc.dram_tensor("qkT_sp", [128, 8, T], bf16)  # scratch: q,k feat-major

    TA = 256            # stage-1 token block
    NBLK = T // TA      # 16

    with tile.TileContext(nc) as tc:
        with tc.tile_pool(name="const", bufs=1) as pconst:
            ones_col = pconst.tile([128, 1], bf16)
            nc.sync.dma_start(out=ones_col[:, :], in_=ones_c[:, :])
            ones_row = pconst.tile([1, 128], bf16)
            nc.sync.dma_start(out=ones_row[:, :], in_=ones_r[:, :])
            bqk_sb = pconst.tile([128, 8], f32)
            nc.sync.dma_start(out=bqk_sb[:, :], in_=bqk[:, :])
            bv_sb = pconst.tile([1, 512], bf16)
            nc.sync.dma_start(out=bv_sb[:, :], in_=bv[:, :])

            from contextlib import ExitStack
            _stk = ExitStack()
            ppersist = _stk.enter_context(tc.tile_pool(name="persist", bufs=1))
            v_sb = ppersist.tile([128, 32, 512], bf16)      # v, token-major
            attnoutT = ppersist.tile([128, HPC, T], bf16)   # lives st2->st3

            def _stages():
              # ------------- stage 1: fused QKV projection, one x pass -------
              if "1" in stages:
               with tc.tile_pool(name="w1", bufs=1) as pw, \
                    tc.tile_pool(name="x1", bufs=2) as px, \
                    tc.tile_pool(name="ev1", bufs=4) as pev, \
                    tc.tile_pool(name="psqk", bufs=4, space="PSUM") as ppsqk, \
                    tc.tile_pool(name="psv", bufs=2, space="PSUM") as ppsv:
                   w_sb = pw.tile([128, 32, 1024], bf16)
                   nc.sync.dma_start(out=w_sb[:, :, :], in_=wqkT[:, :, :])
                   wv_sb = pw.tile([128, 32, 512], bf16)
                   nc.sync.dma_start(out=wv_sb[:, :, :], in_=wvT[:, :, :])
                   for tb in range(NBLK):
                       x_sb = px.tile([128, 32, TA], bf16, tag="x1")
                       nc.sync.dma_start(
                           out=x_sb[:, :, :], in_=xT[:, :, TA * tb:TA * (tb + 1)]
                       )
                       # q,k: feature-major [qk-dim 128, tokens TA]
                       for ji in range(8):  # 4 q head-slots then 4 k head-slots
                           ps = ppsqk.tile([128, TA], f32, tag="psqk")
                           for h in range(32):
                               nc.tensor.matmul(
                                   ps[:, :],
                                   w_sb[:, h, 128 * ji:128 * (ji + 1)],
                                   x_sb[:, h, :],
                                   start=(h == 0), stop=(h == 31),
                               )
                           ev = pev.tile([128, TA], bf16, tag="ev_qk")
                           nc.scalar.activation(
                               ev[:, :], ps[:, :], AF.Identity,
                               bias=bqk_sb[:, ji:ji + 1],
                           )
                           nc.sync.dma_start(
                               out=qkT_sp[:, ji, TA * tb:TA * (tb + 1)],
                               in_=ev[:, :],
                           )
                       # v: token-major [tokens 128, vcols 512]
                       for sub in range(TA // 128):
                           tt = (TA * tb) // 128 + sub
                           psv = ppsv.tile([128, 512], f32, tag="psv")
                           nc.tensor.matmul(  # bias row outer product
                               psv[:, :], ones_row[:, :], bv_sb[:, :],
                               start=True, stop=False,
                           )
                           for h in range(32):
                               nc.tensor.matmul(
                                   psv[:, :],
                                   x_sb[:, h, 128 * sub:128 * (sub + 1)],
                                   wv_sb[:, h, :],
                                   start=False, stop=(h == 31),
                               )
                           nc.vector.tensor_copy(v_sb[:, tt, :], psv[:, :])

              # ------------- stage 2: attention (+ dense interleaved per b) --
              if "2" in stages:
               with tc.tile_pool(name="wd3", bufs=1) as pwd, \
                    tc.tile_pool(name="o3", bufs=4) as po, \
                    tc.tile_pool(name="ps3", bufs=2, space="PSUM") as pp3, \
                    tc.tile_pool(name="qk2", bufs=1) as pqk, \
                    tc.tile_pool(name="bt2", bufs=2) as pbt, \
                    tc.tile_pool(name="p2", bufs=8) as pp, \
                    tc.tile_pool(name="bc2", bufs=1) as pbc, \
                    tc.tile_pool(name="rr2", bufs=1) as prr, \
                    tc.tile_pool(name="pso", bufs=2, space="PSUM") as ppo, \
                    tc.tile_pool(name="psr", bufs=1, space="PSUM") as ppr, \
                    tc.tile_pool(name="pss", bufs=3, space="PSUM") as pp_s:
                  wd_sb = pwd.tile([128, HPC, H], bf16)
                  nc.sync.dma_start(out=wd_sb[:, :, :], in_=wdT[:, :, :])
                  for b in range(B):
                      qk_sb = pqk.tile([128, 8, S], bf16, tag="qk_sb")
                      nc.sync.dma_start(
                          out=qk_sb[:, :, :], in_=qkT_sp[:, :, S * b:S * (b + 1)]
                      )
                      for j in range(HPC):
                          bt_sb = pbt.tile([128, 16, 512], bf16, tag="bt_sb")
                          nc.sync.dma_start(out=bt_sb[:, :, :], in_=btil[j, :, :, :])
                          for q0 in range(4):
                              keep = _keep_k_tiles(j, q0)
                              out_ps = ppo.tile([128, 512], f32, tag="out_ps")
                              rs_ps = ppr.tile([1, 512], f32, tag="rs_ps")
                              # software pipeline: run scores+exp+mask LA tiles
                              # ahead so the PE never waits on ACT/DVE
                              LA = 3
                              pending = []

                              def _produce(kt):
                                  s_ps = pp_s.tile([128, 512], f32, tag="s_ps")
                                  nc.tensor.matmul(
                                      s_ps[:, :],
                                      qk_sb[:, 4 + j, 128 * kt:128 * (kt + 1)],
                                      qk_sb[:, j, 512 * q0:512 * (q0 + 1)],
                                      start=True, stop=True,
                                  )
                                  p0 = pp.tile([128, 512], bf16, tag="p0")
                                  nc.scalar.activation(p0[:, :], s_ps[:, :], AF.Exp)
                                  pT = pp.tile([128, 512], bf16, tag="pT")
                                  nc.vector.tensor_tensor(
                                      pT[:, :], p0[:, :],
                                      bt_sb[:, kt - 4 * q0 + 12, :], op=MULT,
                                  )
                                  return pT

                              def _consume(i, kt, pT):
                                  first, last = (i == 0), (i == len(keep) - 1)
                                  nc.tensor.matmul(
                                      out_ps[:, :],
                                      v_sb[:, 16 * b + kt, 128 * j:128 * (j + 1)],
                                      pT[:, :], start=first, stop=last,
                                  )
                                  nc.tensor.matmul(
                                      rs_ps[:, :], ones_col[:, :], pT[:, :],
                                      start=first, stop=last,
                                  )

                              for i, kt in enumerate(keep):
                                  pending.append((i, kt, _produce(kt)))
                                  if len(pending) > LA:
                                      _consume(*pending.pop(0))
                              for item in pending:
                                  _consume(*item)
                              rr = prr.tile([1, 512], bf16, tag="rr")
                              with nc.allow_low_precision(
                                      reason="softmax denom; 0.4% uniform"):
                                  nc.vector.reciprocal(rr[:, :], rs_ps[:, :])
                              bc_ps = pp_s.tile([128, 512], f32, tag="s_ps")
                              nc.tensor.matmul(
                                  bc_ps[:, :], ones_row[:, :], rr[:, :],
                                  start=True, stop=True,
                              )
                              bc_sb = pbc.tile([128, 512], f32, tag="bc_sb")
                              nc.scalar.copy(bc_sb[:, :], bc_ps[:, :])
                              tpos = S * b + 512 * q0
                              nc.vector.tensor_tensor(
                                  attnoutT[:, j, tpos:tpos + 512],
                                  out_ps[:, :], bc_sb[:, :], op=MULT,
                              )
                      # ---- stage 3: dense (partial over this core's heads),
                      # emitted per batch so its MMs fill the PE stream ----
                      for tt in range(16 * b, 16 * (b + 1)):
                          for ob in range(8):
                              ps3 = pp3.tile([128, 512], f32, tag="ps3")
                              for j in range(HPC):
                                  nc.tensor.matmul(
                                      ps3[:, :],
                                      attnoutT[:, j, 128 * tt:128 * (tt + 1)],
                                      wd_sb[:, j, 512 * ob:512 * (ob + 1)],
                                      start=(j == 0), stop=(j == HPC - 1),
                                  )
                              o_sb = po.tile([128, 512], bf16, tag="o_sb")
                              nc.scalar.copy(o_sb[:, :], ps3[:, :])
                              nc.sync.dma_start(
                                  out=out[128 * tt:128 * (tt + 1),
                                          512 * ob:512 * (ob + 1)],
                                  in_=o_sb[:, :],
                              )

            if trips == 1:
                _stages()
            else:
                with tc.For_i(0, trips, 1):
                    _stages()
            _stk.close()

    nc.compile()
    return nc


_NC_CACHE = None


def _get_program():
    global _NC_CACHE
    if _NC_CACHE is None:
        _NC_CACHE = _build_program()
    return _NC_CACHE


def _feature_major(w_rows: np.ndarray) -> np.ndarray:
    # [512, H] weight rows -> [128, 32, 512] contraction-major tiles
    return np.ascontiguousarray(
        w_rows.T.reshape(32, 128, 512).transpose(1, 0, 2)
    )


def make_core_inputs(hidden_states, w_qkv, b_qkv, w_dense):
    """Shard + relayout full inputs into the 8 per-core input maps."""
    x = np.asarray(hidden_states, dtype=F32).reshape(T, H)
    xT_full = np.ascontiguousarray(
        x.T.reshape(32, 128, T).transpose(1, 0, 2)
    ).astype(BF16)
    w_qkv = np.asarray(w_qkv, dtype=F32)
    b_qkv = np.asarray(b_qkv, dtype=F32)
    w_dense = np.asarray(w_dense, dtype=F32)

    in_maps = []
    for c in range(N_CORES):
        heads = heads_for_core(c)
        rows = np.concatenate([np.arange(HD * h, HD * (h + 1)) for h in heads])
        wq_scaled = w_qkv[rows] * SCALE  # fold 1/sqrt(HD) into q
        wqk = np.ascontiguousarray(
            np.concatenate([wq_scaled, w_qkv[H + rows]]).T
            .reshape(32, 128, 1024).transpose(1, 0, 2)).astype(BF16)
        wv = _feature_major(w_qkv[2 * H + rows]).astype(BF16)
        bq = (b_qkv[rows] * SCALE).reshape(HPC, 128)
        bk = b_qkv[H + rows].reshape(HPC, 128)
        bqk_c = np.stack([*bq, *bk], axis=1).astype(F32)  # [128, 8]
        bv_c = b_qkv[2 * H + rows].reshape(1, 512).astype(BF16)

        btil_c = np.zeros((HPC, 128, 16, 512), dtype=BF16)
        kl = np.arange(128)[:, None]
        ql = np.arange(512)[None, :]
        for j, h in enumerate(heads):
            slope = _SLOPES[h]
            for idx in range(16):
                relv = (128 * (idx - 12) + kl - ql).astype(np.float64)
                tilev = np.where(relv <= 0, np.exp(slope * relv), 0.0)
                btil_c[j, :, idx, :] = tilev.astype(BF16)

        wd_c = np.stack(
            [np.ascontiguousarray(w_dense[:, HD * h:HD * (h + 1)].T) for h in heads],
            axis=1,
        ).astype(BF16)  # [128, HPC, H]

        in_maps.append({
            "ones_r": np.ones((1, 128), dtype=BF16),
            "ones_c": np.ones((128, 1), dtype=BF16),
            "xT": xT_full,
            "wqkT": wqk, "wvT": wv,
            "bqk": bqk_c, "bv": bv_c,
            "btil": btil_c, "wdT": wd_c,
        })
    return in_maps


def kernel(hidden_states, w_qkv, b_qkv, w_dense, b_dense):
    from concourse.bass_utils import run_bass_kernel_spmd

    nc = _get_program()
    in_maps = make_core_inputs(hidden_states, w_qkv, b_qkv, w_dense)
    res = run_bass_kernel_spmd(nc, in_maps, core_ids=list(range(N_CORES)))
    acc = np.zeros((T, H), dtype=np.float32)
    for c in range(N_CORES):
        acc += res.results[c]["out"].astype(np.float32)
    acc += np.asarray(b_dense, dtype=np.float32)[None, :]
    return acc.reshape(B, S, H).astype(np.float32)
